# revision 57
# baseline (speedup 1.0000x reference)
"""Multi-head attention (B=32, S=512, D=768, H=12, E=64) on 8 Trainium2 cores.

Sharding: data-parallel over batch — each of the 8 cores processes 4 batches
with a full copy of the weights. No collectives.

v3 design notes (on top of v2's interleaved-pipeline skeleton):

  - Head-PAIR concurrency on the PE via array tiling. Scores (K=E=64) for
    heads (2p, 2p+1) live on disjoint partition halves of qT/kT, so their
    matmuls auto-derive row tile_positions (0,0)/(64,0); AV (M=E=64) outputs
    land on disjoint PSUM halves, auto-deriving col positions (0,0)/(0,64).
    Emitting the pair's matmuls ADJACENT AND INTERLEAVED (A0,B0,A1,B1 — the
    PE dispatches in order, so A0,A1,B0,B1 would serialize on tile A) makes
    them run concurrently (trace shows the B member at ~4ns dur): ~2x
    effective throughput for scores+AV, and the uniform tile modes within a
    block avoid the per-matmul array-mode-switch drains v2 paid
    (scores(64,128)->r(128,32)->AV(128,64) thrash, ~150ns per switch).
  - Softmax denominator r via a broadcast-ones matmul: stationary = [128,64]
    all-ones, so PSUM rows 0:64 all receive r (the partition-broadcast v2
    did with a DRAM-bounce DMA now falls out of the matmul for free). Head
    pairs are col-tiled into one PSUM bank (h even -> partitions 0:64, h odd
    -> 64:128), one DVE reciprocal_approx_fast [128,512] per pair gives 1/r
    on all partitions in SBUF directly. No DRAM bounce, no sync-queue
    traffic, half the reciprocals, and ONE fused normalize-evict DVE mul per
    pair (v2: per head). r shares the (128,64) tile mode with AV: no drain
    between them.
  - PSUM (8 banks): sc 2x(2 banks) + av 1 + r 1 + work 2. The per-pair
    scores are emitted in two j-chunks split around r/AV so the sc pool's
    bufs=2 round-robin never waits on an exp still in flight.
  - Prologue DMA: ALL scattered [c p h e] weight tensors (Wq, Wk, Wv) ride
    the SWDGE/gpsimd queue with the f32->bf16 cast in flight — it sustains
    ~1.2us/chunk where sync/HWDGE degrades to 2-5us under HBM contention
    (every split-across-queues variant measured worse). Queue order x(0),
    Wq, Wk, Wv matches consumer order tr, Q, K, V. The sync queue carries
    only the tiny biases, a staged copy of x(1) (HWDGE can't cast; DVE
    does), the f32 bias broadcasts, and the contiguous Wo. Batch-1's
    transposes run IN THE PROLOGUE between tr(0) and Q-proj, filling the
    gap until Wq lands; batch-1's v/q/k units interleave into
    attention(0). xnat bufs=8 (two batches of x tiles in flight; fewer
    serializes x loads behind their own consumers, ~7us/run). Starting
    attention(0) even earlier with k/v units as pops measures WORSE (the
    in-order PE queue then stalls on late weight DMAs inside attention).

Per-batch PE streaming: transposes 3072cy + QK 36864 + V 18432 + scores
12288 (paired) + r 6144 (paired) + AV 12288 (paired) + out-proj 18432
~= 107.5k cycles ~= 45us/batch; PE busy measured ~212us total (~85%),
steady-state PE idle <0.5us/batch, scores pair-overlap 95/96 (the shared
[A|B] sc tiles), ScalarE exp ~101us, DVE ~166us. Measured 244.1-244.3us
traced on a settled device (sustained back-to-back runs downclock PE
2.4->2.0GHz, +40us — mind the thermal state when comparing) vs v2's
330-374us; rel_err 5.012e-3 (unchanged from v2 — all numerics identical).
"""

import numpy as np

import concourse.bass as bass
import concourse.tile as tile
import concourse.mybir as mybir
from concourse import bacc
from concourse import bass_utils
from concourse.masks import make_identity

B, S, D, H, E = 32, 512, 768, 12, 64
NCORES = 8
BL = B // NCORES          # local batches per core
CD = D // 128             # 6 chunks of 128 over d / he
NP = H // 2               # head pairs per batch
F32 = mybir.dt.float32
BF16 = mybir.dt.bfloat16
AF = mybir.ActivationFunctionType


def build_nc():
    nc = bacc.Bacc(None)

    x_d = nc.dram_tensor("x", [BL, S, D], F32, kind="ExternalInput")
    wq_d = nc.dram_tensor("Wq", [H, D, E], F32, kind="ExternalInput")
    wk_d = nc.dram_tensor("Wk", [H, D, E], F32, kind="ExternalInput")
    wv_d = nc.dram_tensor("Wv", [H, D, E], F32, kind="ExternalInput")
    bq_d = nc.dram_tensor("bq", [H, E], F32, kind="ExternalInput")
    bk_d = nc.dram_tensor("bk", [H, E], F32, kind="ExternalInput")
    bv_d = nc.dram_tensor("bv", [H, E], F32, kind="ExternalInput")
    wo_d = nc.dram_tensor("Wo", [D, D], F32, kind="ExternalInput")
    bo_d = nc.dram_tensor("bo", [D], F32, kind="ExternalInput")
    out_d = nc.dram_tensor("out", [BL, S, D], F32, kind="ExternalOutput")

    with nc.allow_low_precision(reason="bf16 intermediates"), \
         tile.TileContext(nc) as tc:
        with (
            tc.tile_pool(name="singles", bufs=1) as singles,
            tc.tile_pool(name="wstage", bufs=6) as wstage_pool,
            tc.tile_pool(name="xnat", bufs=8) as xnat_pool,
            tc.tile_pool(name="xt", bufs=2) as xt_pool,
            tc.tile_pool(name="qk", bufs=2) as qk_pool,
            tc.tile_pool(name="vv", bufs=2) as v_pool,
            tc.tile_pool(name="ot", bufs=2) as ot_pool,
            tc.tile_pool(name="pt", bufs=12) as pt_pool,
            tc.tile_pool(name="sums", bufs=12) as sums_pool,
            tc.tile_pool(name="bc", bufs=2) as bc_pool,
            tc.tile_pool(name="ostage", bufs=2) as out_pool,
            # PSUM budget (16KB/partition = 8 banks):
            #   sc 2x4KB + av 1x2KB + r 1x2KB + pps 2x2KB
            tc.tile_pool(name="pps", bufs=2, space="PSUM") as pps,
            tc.tile_pool(name="sc_ps", bufs=2, space="PSUM") as sc_ps,
            tc.tile_pool(name="av_ps", bufs=1, space="PSUM") as av_ps,
            tc.tile_pool(name="r_ps", bufs=1, space="PSUM") as r_ps,
        ):
            # ---- constants ----
            ident = singles.tile([128, 128], BF16, tag="ident")
            make_identity(nc, ident)
            # all-ones stationary: r-matmul broadcasts the denominator to
            # 64 PSUM partitions (out[m,q] = sum_k s[k,q] for every m)
            ones64 = singles.tile([128, 64], BF16, tag="ones64")
            nc.vector.memset(ones64, 1.0)

            # ---- x(0) first (SWDGE f32->bf16 cast), then bias broadcasts
            # on the same queue ----
            xn_tiles = {}

            def emit_x_load(b):
                tl = []
                for t4 in range(4):
                    xn = xnat_pool.tile([128, D], BF16)
                    nc.gpsimd.dma_start(
                        out=xn, in_=x_d.ap()[b, t4 * 128:(t4 + 1) * 128, :]
                    )
                    tl.append(xn)
                xn_tiles[b] = tl

            bv_bc = singles.tile([128, D], F32, tag="bvbc")
            bo_bc = singles.tile([128, D], F32, tag="bobc")

            # ---- weights; Wq first so Q-proj starts ASAP. Each of Wq/Wk is
            # split across BOTH DMA queues (sync/HWDGE stages f32 + DVE
            # cast; gpsimd/SWDGE casts f32->bf16 in flight) to halve its
            # landing time; Wv rides SWDGE after them, Wo + the f32 bias
            # broadcasts go on sync. Nothing else sits ahead of x(1) on the
            # SWDGE queue. ----
            w_sb = {}
            bq_sb = singles.tile([128, CD], F32, tag="bq")
            bk_sb = singles.tile([128, CD], F32, tag="bk")

            def load_w_split(name, wd, n_sync):
                t = singles.tile([128, CD, D], BF16, tag=f"w{name}")
                src = wd.ap().rearrange("h (c p) e -> c p h e", p=128)
                for c in range(n_sync):
                    stg = wstage_pool.tile([128, D], F32)
                    nc.sync.dma_start(
                        out=stg.rearrange("p (h e) -> p h e", e=E), in_=src[c]
                    )
                    nc.vector.tensor_copy(out=t[:, c, :], in_=stg)
                for c in range(n_sync, CD):
                    nc.gpsimd.dma_start(
                        out=t[:, c, :].rearrange("p (h e) -> p h e", e=E),
                        in_=src[c],
                    )
                w_sb[name] = t

            nc.sync.dma_start(
                out=bq_sb, in_=bq_d.ap().flatten().rearrange("(m p) -> p m", p=128)
            )
            nc.sync.dma_start(
                out=bk_sb, in_=bk_d.ap().flatten().rearrange("(m p) -> p m", p=128)
            )
            # x(0) stays on SWDGE (it gates the first PE work — staging it
            # through sync measured first-MM at 17.3us vs 11.0). x(1) is
            # staged through the otherwise-idle sync queue (HWDGE can't
            # cast; DVE does) so batch-1 transposes can run in the
            # prologue gap between tr(0) finishing and Wq landing.
            emit_x_load(0)
            x1_tl = []
            for t4 in range(4):
                stg = wstage_pool.tile([128, D], F32)
                nc.sync.dma_start(
                    out=stg, in_=x_d.ap()[1, t4 * 128:(t4 + 1) * 128, :]
                )
                xn = xnat_pool.tile([128, D], BF16)
                nc.vector.tensor_copy(out=xn, in_=stg)
                x1_tl.append(xn)
            xn_tiles[1] = x1_tl
            # All three scattered [c p h e] weight tensors ride SWDGE: it
            # sustains ~1.2us/chunk where sync/HWDGE degrades to 3-5us
            # under HBM contention (a late sync Wk chunk was stalling
            # K-proj ~5us). Queue order x0, Wq, Wk, Wv matches consumer
            # order (tr, Q, K, V); sync carries only the contiguous Wo +
            # tiny biases.
            load_w_split("q", wq_d, 0)
            # Wk stays all-SWDGE: splitting 3/3 onto sync re-introduces
            # the scattered-chunk slowness there (measured q0 idle 22.7us
            # vs 16.3, gaps up to 5.8us) even with sync otherwise idle.
            load_w_split("k", wk_d, 0)
            load_w_split("v", wv_d, 0)
            for dst, src_d in ((bv_bc, bv_d), (bo_bc, bo_d)):
                f = src_d.ap().flatten()
                nc.sync.dma_start(
                    out=dst,
                    in_=bass.AP(tensor=f.tensor, offset=f.offset,
                                ap=[[0, 128]] + [list(p) for p in f.ap]),
                )
            # Wo rides the SWDGE tail (bf16 cast in flight, after Wv): it
            # isn't needed until the first out-proj (~60us), and its sync
            # staging traffic was contending with SWDGE's Wq/Wk delivery
            # during the critical 10-30us prologue window.
            wo_sb = singles.tile([128, CD, D], BF16, tag="wo")
            wo_src = wo_d.ap().rearrange("(c p) n -> c p n", p=128)
            for c in range(CD):
                nc.gpsimd.dma_start(out=wo_sb[:, c, :], in_=wo_src[c])

            xt_tiles = {}
            qk_tiles = {}
            v_tiles = {}
            ot_tiles = {}

            def p1_units(b, defer_tr=False):
                """The next batch's prep as a list of PE work units, to be
                interleaved between attention pairs of the current batch."""
                units = []
                xt = xt_pool.tile([128, CD, S], BF16)
                xt_tiles[b] = xt

                def transpose_unit(t4, cg, ncg):
                    def emit():
                        xn = xn_tiles[b][t4]
                        tp = pps.tile([128, S], BF16, tag="ps")
                        for j in range(ncg):
                            c = cg + j
                            nc.tensor.transpose(
                                tp[:, j * 128:(j + 1) * 128],
                                xn[:, c * 128:(c + 1) * 128],
                                ident,
                            )
                        nc.vector.tensor_copy(
                            out=xt[:, cg:cg + ncg, t4 * 128:(t4 + 1) * 128],
                            in_=tp[:, 0:ncg * 128].rearrange(
                                "p (c q) -> p c q", q=128),
                        )
                    return emit

                tr_list = [transpose_unit(t4, cg, ncg)
                           for t4 in range(4) for cg, ncg in ((0, 4), (4, 2))]
                if not defer_tr:
                    units += tr_list

                v_units = []
                v_sb = v_pool.tile([128, 4, D], BF16)
                v_tiles[b] = v_sb

                def v_unit(t4, n):
                    def emit():
                        ps = pps.tile([128, S], F32, tag="ps")
                        for c in range(CD):
                            nc.tensor.matmul(
                                ps[:, 0:384],
                                lhsT=xt[:, c, t4 * 128:(t4 + 1) * 128],
                                rhs=w_sb["v"][:, c, n * 384:(n + 1) * 384],
                                start=(c == 0),
                                stop=(c == CD - 1),
                            )
                        nc.vector.tensor_add(
                            out=v_sb[:, t4, n * 384:(n + 1) * 384],
                            in0=ps[:, 0:384],
                            in1=bv_bc[:, n * 384:(n + 1) * 384],
                        )
                    return emit

                for t4 in range(4):
                    for n in range(2):
                        v_units.append(v_unit(t4, n))

                qT = qk_pool.tile([128, CD, S], BF16, tag="qT")
                kT = qk_pool.tile([128, CD, S], BF16, tag="kT")
                qk_tiles[b] = (qT, kT)

                def qk_unit(dst, wname, bsb, m):
                    def emit():
                        ps = pps.tile([128, S], F32, tag="ps")
                        for c in range(CD):
                            nc.tensor.matmul(
                                ps,
                                lhsT=w_sb[wname][:, c, m * 128:(m + 1) * 128],
                                rhs=xt_tiles[b][:, c, :],
                                start=(c == 0),
                                stop=(c == CD - 1),
                            )
                        # eviction on DVE: ScalarE must stay exp-only, else
                        # exp-B of each pair runs late, its sc tile frees
                        # late, and the next chunk's B matmul loses the
                        # row-tile overlap (observed as [319,216,318,216]
                        # serial chunks instead of [319,3,213,3])
                        nc.vector.tensor_scalar_add(
                            out=dst[:, m, :], in0=ps, scalar1=bsb[:, m:m + 1],
                        )
                    return emit

                q_units = [qk_unit(qT, "q", bq_sb, m) for m in range(CD)]
                k_units = [qk_unit(kT, "k", bk_sb, m) for m in range(CD)]
                if b == 0:
                    # prologue: Wq lands first, Wk second, Wv third — order
                    # the GEMMs to chase the weight DMAs. (Starting
                    # attention(0) earlier with k/v units as interleave pops
                    # measures WORSE: the in-order PE queue then stalls on
                    # late weight DMAs inside the attention chain.)
                    units += q_units + k_units + v_units
                    return units, [], tr_list
                # steady state: V(t4) only needs its own t4 transposed;
                # QK needs the full xt. For the LAST batch, its qk m=4,5
                # units are held back into its OWN attention's interleave —
                # otherwise attention(BL-1) has only 4 units of cover, its
                # iterations compress, ScalarE exp falls behind and the
                # scores pairs serialize. (scores(4) consumes m=4 only at
                # iteration 2; the held units pop in the prologue.)
                units += v_units
                if b == BL - 1:
                    units += q_units[:4] + k_units[:4]
                    return units, q_units[4:] + k_units[4:], tr_list
                units += q_units + k_units
                return units, [], tr_list

            def emit_attention(b, interleave):
                """Per head pair p (heads 2p, 2p+1 on partition halves of
                qT/kT chunk p): row-tiled concurrent scores -> exp (ScalarE,
                two-block tiles) -> pair-sums (DVE, bf16) -> col-tiled
                broadcast-ones r matmuls (one PSUM bank, 1/r on all 128
                partitions after one DVE reciprocal) -> col-tiled concurrent
                AV -> ONE fused normalize-evict DVE mul per pair. Scores run
                2 pairs ahead, emitted in two j-chunks around r/AV;
                `interleave` units are popped between pairs."""
                qT, kT = qk_tiles[b]
                v_sb = v_tiles[b]
                oT = ot_pool.tile([128, CD, S], BF16, tag="oT")
                ot_tiles[b] = oT

                pt_tiles = {}
                sum_tiles = {}
                bc_tiles = {}

                def emit_scores_chunk(p, j):
                    """j-chunk (key blocks 2j, 2j+1) of scores for heads
                    (2p, 2p+1), interleaved A0,B0,A1,B1 for row-tile
                    concurrency. Tile X_i holds [A's block | B's block] in
                    its two banks, so the sc-pool recycle wait (exp of two
                    chunks back) gates BOTH pair members of an i-step
                    together — per-head tiles freed B's tile late (exp-B
                    runs after exp-A on ScalarE) and serialized B's matmul,
                    losing the pair overlap ~half the time (~10us/run).
                    (A 64x64 four-way col-split variant measures WORSE:
                    8 instruction issues + LDWs outweigh the concurrency.)"""
                    hA, hB = 2 * p, 2 * p + 1
                    scs = [sc_ps.tile([128, 2, S], F32, tag="sc", name="sc")
                           for _ in (0, 1)]
                    for i in (0, 1):
                        t4 = 2 * j + i
                        t4s = slice(t4 * 128, (t4 + 1) * 128)
                        for sel, h in ((0, hA), (1, hB)):
                            half = 64 * (h % 2)
                            nc.tensor.matmul(
                                scs[i][:, sel, :],
                                lhsT=kT[half:half + 64, p, t4s],
                                rhs=qT[half:half + 64, p, :],
                                start=True, stop=True,
                            )
                    pts = []
                    for i in (0, 1):
                        pt = pt_pool.tile([128, 2, S], BF16)
                        nc.scalar.activation(
                            out=pt.rearrange("p a b -> p (a b)"),
                            in_=scs[i].rearrange("p a b -> p (a b)"),
                            func=AF.Exp, scale=0.125)
                        pts.append(pt)
                    pt_tiles[(p, j)] = pts
                    for sel, h in ((0, hA), (1, hB)):
                        s = sums_pool.tile([128, S], BF16, tag="s16")
                        nc.vector.tensor_add(out=s, in0=pts[0][:, sel, :],
                                             in1=pts[1][:, sel, :])
                        sum_tiles.setdefault(h, [None, None])[j] = s

                def emit_r(p):
                    """Col-tiled pair: head 2p -> PSUM partitions 0:64,
                    head 2p+1 -> 64:128, each row r-broadcast. Interleaved
                    for tile concurrency."""
                    rp = r_ps.tile([128, S], F32, tag="rp")
                    hA, hB = 2 * p, 2 * p + 1
                    for j in (0, 1):
                        nc.tensor.matmul(
                            rp[0:64, :], lhsT=ones64, rhs=sum_tiles[hA][j],
                            start=(j == 0), stop=(j == 1),
                        )
                        nc.tensor.matmul(
                            rp[64:128, :], lhsT=ones64, rhs=sum_tiles[hB][j],
                            start=(j == 0), stop=(j == 1),
                        )
                    bc = bc_pool.tile([128, S], F32, tag="bc")
                    nc.vector.reciprocal_approx_fast(out=bc, in_=rp)
                    bc_tiles[p] = bc
                    del sum_tiles[hA], sum_tiles[hB]

                def emit_av(p):
                    """Col-tiled concurrent AV for the pair into one PSUM
                    bank (h even -> partitions 0:64, h odd -> 64:128), then
                    one fused normalize-evict DVE mul."""
                    av = av_ps.tile([128, S], F32, tag="av")
                    hA, hB = 2 * p, 2 * p + 1
                    for t4 in range(4):
                        for sel, (h, half) in ((0, (hA, 0)), (1, (hB, 64))):
                            nc.tensor.matmul(
                                av[half:half + 64, :],
                                lhsT=v_sb[:, t4, h * 64:(h + 1) * 64],
                                rhs=pt_tiles[(p, t4 // 2)][t4 % 2][:, sel, :],
                                start=(t4 == 0), stop=(t4 == 3),
                            )
                    nc.vector.tensor_mul(
                        out=oT[:, p, :], in0=av, in1=bc_tiles[p],
                    )
                    del pt_tiles[(p, 0)], pt_tiles[(p, 1)], bc_tiles[p]

                # Adaptive pop pacing: spread the interleave units evenly
                # over the remaining pop points. Front-loading (3+2) runs
                # the list dry by iteration ~4; the j0->j1 chunk spacing
                # then drops below the ~2.5us sc-tile recycle latency
                # (exp of the chunk before last) and the scores pairs lose
                # their row-tile overlap (B serializes, ~10us/run).
                # one phantom point: ~1/15 of the units survive the loop
                # and run at the batch boundary, covering the next batch's
                # first-scores wait on the sc-tile recycle (~1.1us/boundary)
                points = [2 + 2 * NP + 1]

                def pop():
                    n = -(-len(interleave) // points[0]) if interleave else 0
                    points[0] -= 1
                    for _ in range(n):
                        if interleave:
                            interleave.pop(0)()

                # prologue: two pairs of scores ahead
                emit_scores_chunk(0, 0)
                emit_scores_chunk(0, 1)
                pop()
                emit_scores_chunk(1, 0)
                emit_scores_chunk(1, 1)
                pop()
                for p in range(NP):
                    if p + 2 < NP:
                        emit_scores_chunk(p + 2, 0)
                    emit_r(p)
                    emit_av(p)
                    pop()
                    if p + 2 < NP:
                        emit_scores_chunk(p + 2, 1)
                    pop()

            def p3_units(b):
                """Out-projection as 4 per-token-block units, interleaved
                into the NEXT batch's attention for PE cover."""
                def t4_unit(t4):
                    def emit():
                        oT = ot_tiles[b]
                        ostage = out_pool.tile([128, D], F32)
                        for n in range(2):
                            ps = pps.tile([128, S], F32, tag="ps")
                            for m in range(CD):
                                nc.tensor.matmul(
                                    ps[:, 0:384],
                                    lhsT=oT[:, m, t4 * 128:(t4 + 1) * 128],
                                    rhs=wo_sb[:, m, n * 384:(n + 1) * 384],
                                    start=(m == 0),
                                    stop=(m == CD - 1),
                                )
                            nc.vector.tensor_add(
                                out=ostage[:, n * 384:(n + 1) * 384],
                                in0=ps[:, 0:384],
                                in1=bo_bc[:, n * 384:(n + 1) * 384],
                            )
                        nc.sync.dma_start(
                            out=out_d.ap()[b, t4 * 128:(t4 + 1) * 128, :],
                            in_=ostage,
                        )
                    return emit
                return [t4_unit(t4) for t4 in range(4)]

            # ---- pipeline ----
            # prologue PE order: tr(0), tr(1) (x(1) came via sync — fills
            # the gap until Wq lands), then Q(0), K(0), V(0) chasing the
            # SWDGE weight chain. Batch 1's v/q/k units interleave into
            # attention(0).
            units0, _, _ = p1_units(0)
            now1, held, tr1 = p1_units(1, defer_tr=True)
            for unit in units0[:8]:
                unit()
            for unit in tr1:
                unit()
            for unit in units0[8:]:
                unit()
            prev_p3 = []
            for b in range(BL):
                head = held + prev_p3
                if b == 0:
                    units = head + now1
                elif b + 1 < BL:
                    emit_x_load(b + 1)
                    now, held, _ = p1_units(b + 1)
                    units = head + now
                else:
                    units = head
                emit_attention(b, units)
                for unit in units:
                    unit()
                prev_p3 = p3_units(b)
            for unit in prev_p3:
                unit()

    nc.finalize()
    return nc


_NC_CACHE = None


def _get_nc():
    global _NC_CACHE
    if _NC_CACHE is None:
        _NC_CACHE = build_nc()
    return _NC_CACHE


def run_spmd(inputs, trace=False, trace_cores=None):
    nc = _get_nc()
    x = np.ascontiguousarray(inputs["x"], dtype=np.float32)
    shared = {
        k: np.ascontiguousarray(inputs[k], dtype=np.float32)
        for k in ("Wq", "Wk", "Wv", "bq", "bk", "bv", "Wo", "bo")
    }
    in_maps = []
    for core in range(NCORES):
        m = dict(shared)
        m["x"] = np.ascontiguousarray(x[core * BL:(core + 1) * BL])
        in_maps.append(m)
    res = bass_utils.run_bass_kernel_spmd(
        nc, in_maps, core_ids=list(range(NCORES)),
        trace=trace, trace_cores=trace_cores,
    )
    return res


def kernel(**inputs) -> np.ndarray:
    res = run_spmd(inputs, trace=False)
    out = np.concatenate([res.results[i]["out"] for i in range(NCORES)], axis=0)
    return out.astype(np.float32)


# revision 59
# speedup vs baseline: 1.0050x; 1.0050x over previous
"""Multi-head attention (B=32, S=512, D=768, H=12, E=64) on 8 Trainium2 cores.

Sharding: data-parallel over batch — each of the 8 cores processes 4 batches
with a full copy of the weights. No collectives.

v3 design notes (on top of v2's interleaved-pipeline skeleton):

  - Head-PAIR concurrency on the PE via array tiling. Scores (K=E=64) for
    heads (2p, 2p+1) live on disjoint partition halves of qT/kT, so their
    matmuls auto-derive row tile_positions (0,0)/(64,0); AV (M=E=64) outputs
    land on disjoint PSUM halves, auto-deriving col positions (0,0)/(0,64).
    Emitting the pair's matmuls ADJACENT AND INTERLEAVED (A0,B0,A1,B1 — the
    PE dispatches in order, so A0,A1,B0,B1 would serialize on tile A) makes
    them run concurrently (trace shows the B member at ~4ns dur): ~2x
    effective throughput for scores+AV, and the uniform tile modes within a
    block avoid the per-matmul array-mode-switch drains v2 paid
    (scores(64,128)->r(128,32)->AV(128,64) thrash, ~150ns per switch).
  - Softmax denominator r via a broadcast-ones matmul: stationary = [128,64]
    all-ones, so PSUM rows 0:64 all receive r (the partition-broadcast v2
    did with a DRAM-bounce DMA now falls out of the matmul for free). Head
    pairs are col-tiled into one PSUM bank (h even -> partitions 0:64, h odd
    -> 64:128), one DVE reciprocal_approx_fast [128,512] per pair gives 1/r
    on all partitions in SBUF directly. No DRAM bounce, no sync-queue
    traffic, half the reciprocals, and ONE fused normalize-evict DVE mul per
    pair (v2: per head). r shares the (128,64) tile mode with AV: no drain
    between them.
  - PSUM (8 banks): sc 2x(2 banks) + av 1 + r 1 + work 2. The per-pair
    scores are emitted in two j-chunks split around r/AV so the sc pool's
    bufs=2 round-robin never waits on an exp still in flight.
  - Prologue DMA: ALL scattered [c p h e] weight tensors (Wq, Wk, Wv) ride
    the SWDGE/gpsimd queue with the f32->bf16 cast in flight — it sustains
    ~1.2us/chunk where sync/HWDGE degrades to 2-5us under HBM contention
    (every split-across-queues variant measured worse). Queue order x(0),
    Wq, Wk, Wv matches consumer order tr, Q, K, V. The sync queue carries
    only the tiny biases, a staged copy of x(1) (HWDGE can't cast; DVE
    does), the f32 bias broadcasts, and the contiguous Wo. Batch-1's
    transposes run IN THE PROLOGUE between tr(0) and Q-proj, filling the
    gap until Wq lands; batch-1's v/q/k units interleave into
    attention(0). xnat bufs=8 (two batches of x tiles in flight; fewer
    serializes x loads behind their own consumers, ~7us/run). Starting
    attention(0) even earlier with k/v units as pops measures WORSE (the
    in-order PE queue then stalls on late weight DMAs inside attention).

Per-batch PE streaming: transposes 3072cy + QK 36864 + V 18432 + scores
12288 (paired) + r 6144 (paired) + AV 12288 (paired) + out-proj 18432
~= 107.5k cycles ~= 45us/batch; PE busy measured ~212us total (~85%),
steady-state PE idle <0.5us/batch, scores pair-overlap 95/96 (the shared
[A|B] sc tiles), ScalarE exp ~101us, DVE ~166us. Measured 244.1-244.3us
traced on a settled device (sustained back-to-back runs downclock PE
2.4->2.0GHz, +40us — mind the thermal state when comparing) vs v2's
330-374us; rel_err 5.012e-3 (unchanged from v2 — all numerics identical).
"""

import numpy as np

import concourse.bass as bass
import concourse.tile as tile
import concourse.mybir as mybir
from concourse import bacc
from concourse import bass_utils
from concourse.masks import make_identity

B, S, D, H, E = 32, 512, 768, 12, 64
NCORES = 8
BL = B // NCORES          # local batches per core
CD = D // 128             # 6 chunks of 128 over d / he
NP = H // 2               # head pairs per batch
F32 = mybir.dt.float32
BF16 = mybir.dt.bfloat16
AF = mybir.ActivationFunctionType


def build_nc():
    nc = bacc.Bacc(None)

    x_d = nc.dram_tensor("x", [BL, S, D], F32, kind="ExternalInput")
    wq_d = nc.dram_tensor("Wq", [H, D, E], F32, kind="ExternalInput")
    wk_d = nc.dram_tensor("Wk", [H, D, E], F32, kind="ExternalInput")
    wv_d = nc.dram_tensor("Wv", [H, D, E], F32, kind="ExternalInput")
    bq_d = nc.dram_tensor("bq", [H, E], F32, kind="ExternalInput")
    bk_d = nc.dram_tensor("bk", [H, E], F32, kind="ExternalInput")
    bv_d = nc.dram_tensor("bv", [H, E], F32, kind="ExternalInput")
    wo_d = nc.dram_tensor("Wo", [D, D], F32, kind="ExternalInput")
    bo_d = nc.dram_tensor("bo", [D], F32, kind="ExternalInput")
    out_d = nc.dram_tensor("out", [BL, S, D], F32, kind="ExternalOutput")

    with nc.allow_low_precision(reason="bf16 intermediates"), \
         tile.TileContext(nc) as tc:
        with (
            tc.tile_pool(name="singles", bufs=1) as singles,
            tc.tile_pool(name="wstage", bufs=6) as wstage_pool,
            tc.tile_pool(name="xnat", bufs=8) as xnat_pool,
            tc.tile_pool(name="xt", bufs=2) as xt_pool,
            tc.tile_pool(name="qk", bufs=2) as qk_pool,
            tc.tile_pool(name="vv", bufs=2) as v_pool,
            tc.tile_pool(name="ot", bufs=2) as ot_pool,
            tc.tile_pool(name="pt", bufs=12) as pt_pool,
            tc.tile_pool(name="sums", bufs=12) as sums_pool,
            tc.tile_pool(name="bc", bufs=2) as bc_pool,
            tc.tile_pool(name="ostage", bufs=2) as out_pool,
            # PSUM budget (16KB/partition = 8 banks):
            #   sc 2x4KB + av 1x2KB + r 1x2KB + pps 2x2KB
            tc.tile_pool(name="pps", bufs=2, space="PSUM") as pps,
            tc.tile_pool(name="sc_ps", bufs=2, space="PSUM") as sc_ps,
            tc.tile_pool(name="av_ps", bufs=1, space="PSUM") as av_ps,
            tc.tile_pool(name="r_ps", bufs=1, space="PSUM") as r_ps,
        ):
            # ---- constants ----
            ident = singles.tile([128, 128], BF16, tag="ident")
            make_identity(nc, ident)
            # all-ones stationary: r-matmul broadcasts the denominator to
            # 64 PSUM partitions (out[m,q] = sum_k s[k,q] for every m)
            ones64 = singles.tile([128, 64], BF16, tag="ones64")
            nc.vector.memset(ones64, 1.0)

            # HAM warm-up: ~40 tiny matmuls fill the otherwise-idle
            # framework-setup/DMA window (~7-11us) so the real transposes
            # and Q-proj start at 2.4GHz instead of paying the ~3.4us
            # cold-clock ramp (first 16 transposes measured 107ns vs 56ns
            # warm). One DVE read afterwards keeps the tile graph clean.
            warm_ps = pps.tile([128, 64], F32, tag="ps", name="warm")
            for _ in range(40):
                nc.tensor.matmul(warm_ps[0:64, :], lhsT=ones64, rhs=ones64,
                                 start=True, stop=True)
            warm_rd = singles.tile([128, 64], BF16, tag="warmrd")
            nc.vector.tensor_copy(out=warm_rd[0:64, :], in_=warm_ps[0:64, :])

            # ---- x(0) first (SWDGE f32->bf16 cast), then bias broadcasts
            # on the same queue ----
            xn_tiles = {}

            def emit_x_load(b):
                tl = []
                for t4 in range(4):
                    xn = xnat_pool.tile([128, D], BF16)
                    nc.gpsimd.dma_start(
                        out=xn, in_=x_d.ap()[b, t4 * 128:(t4 + 1) * 128, :]
                    )
                    tl.append(xn)
                xn_tiles[b] = tl

            bv_bc = singles.tile([128, D], F32, tag="bvbc")
            bo_bc = singles.tile([128, D], F32, tag="bobc")

            # ---- weights; Wq first so Q-proj starts ASAP. Each of Wq/Wk is
            # split across BOTH DMA queues (sync/HWDGE stages f32 + DVE
            # cast; gpsimd/SWDGE casts f32->bf16 in flight) to halve its
            # landing time; Wv rides SWDGE after them, Wo + the f32 bias
            # broadcasts go on sync. Nothing else sits ahead of x(1) on the
            # SWDGE queue. ----
            w_sb = {}
            bq_sb = singles.tile([128, CD], F32, tag="bq")
            bk_sb = singles.tile([128, CD], F32, tag="bk")

            def load_w_split(name, wd, n_sync):
                t = singles.tile([128, CD, D], BF16, tag=f"w{name}")
                src = wd.ap().rearrange("h (c p) e -> c p h e", p=128)
                for c in range(n_sync):
                    stg = wstage_pool.tile([128, D], F32)
                    nc.sync.dma_start(
                        out=stg.rearrange("p (h e) -> p h e", e=E), in_=src[c]
                    )
                    nc.vector.tensor_copy(out=t[:, c, :], in_=stg)
                for c in range(n_sync, CD):
                    nc.gpsimd.dma_start(
                        out=t[:, c, :].rearrange("p (h e) -> p h e", e=E),
                        in_=src[c],
                    )
                w_sb[name] = t

            nc.sync.dma_start(
                out=bq_sb, in_=bq_d.ap().flatten().rearrange("(m p) -> p m", p=128)
            )
            nc.sync.dma_start(
                out=bk_sb, in_=bk_d.ap().flatten().rearrange("(m p) -> p m", p=128)
            )
            # x(0) stays on SWDGE (it gates the first PE work — staging it
            # through sync measured first-MM at 17.3us vs 11.0). x(1) is
            # staged through the otherwise-idle sync queue (HWDGE can't
            # cast; DVE does) so batch-1 transposes can run in the
            # prologue gap between tr(0) finishing and Wq landing.
            emit_x_load(0)
            x1_tl = []
            for t4 in range(4):
                stg = wstage_pool.tile([128, D], F32)
                nc.sync.dma_start(
                    out=stg, in_=x_d.ap()[1, t4 * 128:(t4 + 1) * 128, :]
                )
                xn = xnat_pool.tile([128, D], BF16)
                nc.vector.tensor_copy(out=xn, in_=stg)
                x1_tl.append(xn)
            xn_tiles[1] = x1_tl
            # All three scattered [c p h e] weight tensors ride SWDGE: it
            # sustains ~1.2us/chunk where sync/HWDGE degrades to 3-5us
            # under HBM contention (a late sync Wk chunk was stalling
            # K-proj ~5us). Queue order x0, Wq, Wk, Wv matches consumer
            # order (tr, Q, K, V); sync carries only the contiguous Wo +
            # tiny biases.
            load_w_split("q", wq_d, 0)
            # Wk stays all-SWDGE: splitting 3/3 onto sync re-introduces
            # the scattered-chunk slowness there (measured q0 idle 22.7us
            # vs 16.3, gaps up to 5.8us) even with sync otherwise idle.
            load_w_split("k", wk_d, 0)
            load_w_split("v", wv_d, 0)
            for dst, src_d in ((bv_bc, bv_d), (bo_bc, bo_d)):
                f = src_d.ap().flatten()
                nc.sync.dma_start(
                    out=dst,
                    in_=bass.AP(tensor=f.tensor, offset=f.offset,
                                ap=[[0, 128]] + [list(p) for p in f.ap]),
                )
            # Wo rides the SWDGE tail (bf16 cast in flight, after Wv): it
            # isn't needed until the first out-proj (~60us), and its sync
            # staging traffic was contending with SWDGE's Wq/Wk delivery
            # during the critical 10-30us prologue window.
            wo_sb = singles.tile([128, CD, D], BF16, tag="wo")
            wo_src = wo_d.ap().rearrange("(c p) n -> c p n", p=128)
            for c in range(CD):
                nc.gpsimd.dma_start(out=wo_sb[:, c, :], in_=wo_src[c])

            xt_tiles = {}
            qk_tiles = {}
            v_tiles = {}
            ot_tiles = {}

            def p1_units(b, defer_tr=False):
                """The next batch's prep as a list of PE work units, to be
                interleaved between attention pairs of the current batch."""
                units = []
                xt = xt_pool.tile([128, CD, S], BF16)
                xt_tiles[b] = xt

                def transpose_unit(t4, cg, ncg):
                    def emit():
                        xn = xn_tiles[b][t4]
                        tp = pps.tile([128, S], BF16, tag="ps")
                        for j in range(ncg):
                            c = cg + j
                            nc.tensor.transpose(
                                tp[:, j * 128:(j + 1) * 128],
                                xn[:, c * 128:(c + 1) * 128],
                                ident,
                            )
                        nc.vector.tensor_copy(
                            out=xt[:, cg:cg + ncg, t4 * 128:(t4 + 1) * 128],
                            in_=tp[:, 0:ncg * 128].rearrange(
                                "p (c q) -> p c q", q=128),
                        )
                    return emit

                tr_list = [transpose_unit(t4, cg, ncg)
                           for t4 in range(4) for cg, ncg in ((0, 4), (4, 2))]
                if not defer_tr:
                    units += tr_list

                v_units = []
                v_sb = v_pool.tile([128, 4, D], BF16)
                v_tiles[b] = v_sb

                def v_unit(t4, n):
                    def emit():
                        ps = pps.tile([128, S], F32, tag="ps")
                        for c in range(CD):
                            nc.tensor.matmul(
                                ps[:, 0:384],
                                lhsT=xt[:, c, t4 * 128:(t4 + 1) * 128],
                                rhs=w_sb["v"][:, c, n * 384:(n + 1) * 384],
                                start=(c == 0),
                                stop=(c == CD - 1),
                            )
                        nc.vector.tensor_add(
                            out=v_sb[:, t4, n * 384:(n + 1) * 384],
                            in0=ps[:, 0:384],
                            in1=bv_bc[:, n * 384:(n + 1) * 384],
                        )
                    return emit

                for t4 in range(4):
                    for n in range(2):
                        v_units.append(v_unit(t4, n))

                qT = qk_pool.tile([128, CD, S], BF16, tag="qT")
                kT = qk_pool.tile([128, CD, S], BF16, tag="kT")
                qk_tiles[b] = (qT, kT)

                def qk_unit(dst, wname, bsb, m):
                    def emit():
                        ps = pps.tile([128, S], F32, tag="ps")
                        for c in range(CD):
                            nc.tensor.matmul(
                                ps,
                                lhsT=w_sb[wname][:, c, m * 128:(m + 1) * 128],
                                rhs=xt_tiles[b][:, c, :],
                                start=(c == 0),
                                stop=(c == CD - 1),
                            )
                        # eviction on DVE: ScalarE must stay exp-only, else
                        # exp-B of each pair runs late, its sc tile frees
                        # late, and the next chunk's B matmul loses the
                        # row-tile overlap (observed as [319,216,318,216]
                        # serial chunks instead of [319,3,213,3])
                        nc.vector.tensor_scalar_add(
                            out=dst[:, m, :], in0=ps, scalar1=bsb[:, m:m + 1],
                        )
                    return emit

                q_units = [qk_unit(qT, "q", bq_sb, m) for m in range(CD)]
                k_units = [qk_unit(kT, "k", bk_sb, m) for m in range(CD)]
                if b == 0:
                    # prologue: Wq lands first, Wk second, Wv third — order
                    # the GEMMs to chase the weight DMAs. (Starting
                    # attention(0) earlier with k/v units as interleave pops
                    # measures WORSE: the in-order PE queue then stalls on
                    # late weight DMAs inside the attention chain.)
                    units += q_units + k_units + v_units
                    return units, [], tr_list
                # steady state: V(t4) only needs its own t4 transposed;
                # QK needs the full xt. For the LAST batch, its qk m=4,5
                # units are held back into its OWN attention's interleave —
                # otherwise attention(BL-1) has only 4 units of cover, its
                # iterations compress, ScalarE exp falls behind and the
                # scores pairs serialize. (scores(4) consumes m=4 only at
                # iteration 2; the held units pop in the prologue.)
                units += v_units
                if b == BL - 1:
                    units += q_units[:4] + k_units[:4]
                    return units, q_units[4:] + k_units[4:], tr_list
                units += q_units + k_units
                return units, [], tr_list

            def emit_attention(b, interleave):
                """Per head pair p (heads 2p, 2p+1 on partition halves of
                qT/kT chunk p): row-tiled concurrent scores -> exp (ScalarE,
                two-block tiles) -> pair-sums (DVE, bf16) -> col-tiled
                broadcast-ones r matmuls (one PSUM bank, 1/r on all 128
                partitions after one DVE reciprocal) -> col-tiled concurrent
                AV -> ONE fused normalize-evict DVE mul per pair. Scores run
                2 pairs ahead, emitted in two j-chunks around r/AV;
                `interleave` units are popped between pairs."""
                qT, kT = qk_tiles[b]
                v_sb = v_tiles[b]
                oT = ot_pool.tile([128, CD, S], BF16, tag="oT")
                ot_tiles[b] = oT

                pt_tiles = {}
                sum_tiles = {}
                bc_tiles = {}

                def emit_scores_chunk(p, j):
                    """j-chunk (key blocks 2j, 2j+1) of scores for heads
                    (2p, 2p+1), interleaved A0,B0,A1,B1 for row-tile
                    concurrency. Tile X_i holds [A's block | B's block] in
                    its two banks, so the sc-pool recycle wait (exp of two
                    chunks back) gates BOTH pair members of an i-step
                    together — per-head tiles freed B's tile late (exp-B
                    runs after exp-A on ScalarE) and serialized B's matmul,
                    losing the pair overlap ~half the time (~10us/run).
                    (A 64x64 four-way col-split variant measures WORSE:
                    8 instruction issues + LDWs outweigh the concurrency.)"""
                    hA, hB = 2 * p, 2 * p + 1
                    scs = [sc_ps.tile([128, 2, S], F32, tag="sc", name="sc")
                           for _ in (0, 1)]
                    for i in (0, 1):
                        t4 = 2 * j + i
                        t4s = slice(t4 * 128, (t4 + 1) * 128)
                        for sel, h in ((0, hA), (1, hB)):
                            half = 64 * (h % 2)
                            nc.tensor.matmul(
                                scs[i][:, sel, :],
                                lhsT=kT[half:half + 64, p, t4s],
                                rhs=qT[half:half + 64, p, :],
                                start=True, stop=True,
                            )
                    pts = []
                    for i in (0, 1):
                        pt = pt_pool.tile([128, 2, S], BF16)
                        nc.scalar.activation(
                            out=pt.rearrange("p a b -> p (a b)"),
                            in_=scs[i].rearrange("p a b -> p (a b)"),
                            func=AF.Exp, scale=0.125)
                        pts.append(pt)
                    pt_tiles[(p, j)] = pts
                    for sel, h in ((0, hA), (1, hB)):
                        s = sums_pool.tile([128, S], BF16, tag="s16")
                        nc.vector.tensor_add(out=s, in0=pts[0][:, sel, :],
                                             in1=pts[1][:, sel, :])
                        sum_tiles.setdefault(h, [None, None])[j] = s

                def emit_r(p):
                    """Col-tiled pair: head 2p -> PSUM partitions 0:64,
                    head 2p+1 -> 64:128, each row r-broadcast. Interleaved
                    for tile concurrency."""
                    rp = r_ps.tile([128, S], F32, tag="rp")
                    hA, hB = 2 * p, 2 * p + 1
                    for j in (0, 1):
                        nc.tensor.matmul(
                            rp[0:64, :], lhsT=ones64, rhs=sum_tiles[hA][j],
                            start=(j == 0), stop=(j == 1),
                        )
                        nc.tensor.matmul(
                            rp[64:128, :], lhsT=ones64, rhs=sum_tiles[hB][j],
                            start=(j == 0), stop=(j == 1),
                        )
                    bc = bc_pool.tile([128, S], F32, tag="bc")
                    nc.vector.reciprocal_approx_fast(out=bc, in_=rp)
                    bc_tiles[p] = bc
                    del sum_tiles[hA], sum_tiles[hB]

                def emit_av(p):
                    """Col-tiled concurrent AV for the pair into one PSUM
                    bank (h even -> partitions 0:64, h odd -> 64:128), then
                    one fused normalize-evict DVE mul."""
                    av = av_ps.tile([128, S], F32, tag="av")
                    hA, hB = 2 * p, 2 * p + 1
                    for t4 in range(4):
                        for sel, (h, half) in ((0, (hA, 0)), (1, (hB, 64))):
                            nc.tensor.matmul(
                                av[half:half + 64, :],
                                lhsT=v_sb[:, t4, h * 64:(h + 1) * 64],
                                rhs=pt_tiles[(p, t4 // 2)][t4 % 2][:, sel, :],
                                start=(t4 == 0), stop=(t4 == 3),
                            )
                    nc.vector.tensor_mul(
                        out=oT[:, p, :], in0=av, in1=bc_tiles[p],
                    )
                    del pt_tiles[(p, 0)], pt_tiles[(p, 1)], bc_tiles[p]

                # Adaptive pop pacing: spread the interleave units evenly
                # over the remaining pop points. Front-loading (3+2) runs
                # the list dry by iteration ~4; the j0->j1 chunk spacing
                # then drops below the ~2.5us sc-tile recycle latency
                # (exp of the chunk before last) and the scores pairs lose
                # their row-tile overlap (B serializes, ~10us/run).
                points = [2 + 2 * NP]

                def pop():
                    n = -(-len(interleave) // points[0]) if interleave else 0
                    points[0] -= 1
                    for _ in range(n):
                        if interleave:
                            interleave.pop(0)()

                # prologue: two pairs of scores ahead
                emit_scores_chunk(0, 0)
                emit_scores_chunk(0, 1)
                pop()
                emit_scores_chunk(1, 0)
                emit_scores_chunk(1, 1)
                pop()
                for p in range(NP):
                    if p + 2 < NP:
                        emit_scores_chunk(p + 2, 0)
                    emit_r(p)
                    emit_av(p)
                    pop()
                    if p + 2 < NP:
                        emit_scores_chunk(p + 2, 1)
                    pop()

            def p3_units(b):
                """Out-projection as 4 per-token-block units, interleaved
                into the NEXT batch's attention for PE cover."""
                def t4_unit(t4):
                    def emit():
                        oT = ot_tiles[b]
                        ostage = out_pool.tile([128, D], F32)
                        for n in range(2):
                            ps = pps.tile([128, S], F32, tag="ps")
                            for m in range(CD):
                                nc.tensor.matmul(
                                    ps[:, 0:384],
                                    lhsT=oT[:, m, t4 * 128:(t4 + 1) * 128],
                                    rhs=wo_sb[:, m, n * 384:(n + 1) * 384],
                                    start=(m == 0),
                                    stop=(m == CD - 1),
                                )
                            nc.vector.tensor_add(
                                out=ostage[:, n * 384:(n + 1) * 384],
                                in0=ps[:, 0:384],
                                in1=bo_bc[:, n * 384:(n + 1) * 384],
                            )
                        nc.sync.dma_start(
                            out=out_d.ap()[b, t4 * 128:(t4 + 1) * 128, :],
                            in_=ostage,
                        )
                    return emit
                return [t4_unit(t4) for t4 in range(4)]

            # ---- pipeline ----
            # prologue PE order: tr(0), tr(1) (x(1) came via sync — fills
            # the gap until Wq lands), then Q(0), K(0), V(0) chasing the
            # SWDGE weight chain. Batch 1's v/q/k units interleave into
            # attention(0).
            units0, _, _ = p1_units(0)
            now1, held, tr1 = p1_units(1, defer_tr=True)
            for unit in units0[:8]:
                unit()
            for unit in tr1:
                unit()
            for unit in units0[8:]:
                unit()
            prev_p3 = []
            for b in range(BL):
                head = held + prev_p3
                if b == 0:
                    units = head + now1
                elif b + 1 < BL:
                    emit_x_load(b + 1)
                    now, held, _ = p1_units(b + 1)
                    units = head + now
                else:
                    units = head
                emit_attention(b, units)
                for unit in units:
                    unit()
                prev_p3 = p3_units(b)
            for unit in prev_p3:
                unit()

    nc.finalize()
    return nc


_NC_CACHE = None


def _get_nc():
    global _NC_CACHE
    if _NC_CACHE is None:
        _NC_CACHE = build_nc()
    return _NC_CACHE


def run_spmd(inputs, trace=False, trace_cores=None):
    nc = _get_nc()
    x = np.ascontiguousarray(inputs["x"], dtype=np.float32)
    shared = {
        k: np.ascontiguousarray(inputs[k], dtype=np.float32)
        for k in ("Wq", "Wk", "Wv", "bq", "bk", "bv", "Wo", "bo")
    }
    in_maps = []
    for core in range(NCORES):
        m = dict(shared)
        m["x"] = np.ascontiguousarray(x[core * BL:(core + 1) * BL])
        in_maps.append(m)
    res = bass_utils.run_bass_kernel_spmd(
        nc, in_maps, core_ids=list(range(NCORES)),
        trace=trace, trace_cores=trace_cores,
    )
    return res


def kernel(**inputs) -> np.ndarray:
    res = run_spmd(inputs, trace=False)
    out = np.concatenate([res.results[i]["out"] for i in range(NCORES)], axis=0)
    return out.astype(np.float32)
